# revision 1
# baseline (speedup 1.0000x reference)
"""BitLinear kernel for Trainium2 (8 NeuronCores, tensor-parallel).

Computes: out = x @ (sign(w) * mean(|w|, axis=1, keepdims=True)).T
  x      : [4, 2048, 4096] f32
  weight : [4096, 4096] f32
  out    : [4, 2048, 4096] f32

Strategy (per sharding hint): shard weight rows (out features) 8-way.
Each core:
  - receives the full activations as x.T in bf16, pre-tiled on host so
    every DMA is a contiguous 256KB chunk with 2KB-per-partition packets
    (xTp [8 pairs, 32 k-tiles, 128, 1024]), plus its weight shard in
    bf16 in natural layout (wn, for the per-row abs-mean scales) and
    k-tile-paired transposed layout (wTt, for the sign operand). bf16
    weights are sign-safe, and the scale's mean over 4096 |w| values
    averages bf16 rounding noise down to ~3e-5 relative.
  - binarizes on device: S = sign(w) in bf16 (exact +-1 / 0); s =
    mean|w| accumulated in f32 by the DVE reduction,
  - matmuls with S as the 128x128 stationary operand and x.T as the
    moving operand ([128, 512] moving tiles), accumulating over k in
    PSUM (f32); the f32 per-feature scale is applied by the scalar
    engine while evicting PSUM -> SBUF; stores write the feature-major
    shard outT [512, 8192].
Host gathers the 8 outT shards -> [4096, 8192] -> transpose -> out.

The matmul stream runs at the PE bf16 fill-rate floor (~216ns per
128x512 matmul). x loads + scale-weight loads are chained in emission
order on the sync engine's DMA queue (startup is HBM-bound and the
order matters); sign-weight loads are interleaved per k-pair; output
stores ride the scalar engine's queue so eviction waits never block x
loads. The first block pair is computed j-outer across all 8 PSUM banks
so the PE keeps pace with the HBM-limited startup stream.
"""

import os
from contextlib import ExitStack

import numpy as np
import ml_dtypes

import concourse.bass as bass
import concourse.mybir as mybir
import concourse.tile as tile
from concourse import bacc, bass_utils

P = 128                 # SBUF partitions / PE array dim
D_IN = 4096             # contraction dim (in features)
D_OUT = 4096            # out features
M_TOT = 8192            # tokens (4*2048)
N_CORES = 8
N_SHARD = D_OUT // N_CORES      # 512 out features per core
K_TILES = D_IN // P             # 32
M_BLK = 512                     # moving free dim per matmul
M_BLKS = M_TOT // M_BLK         # 16
M_PAIRS = M_BLKS // 2           # 8 (x is loaded in block pairs)
N_TILES = N_SHARD // P          # 4

_CACHE = {}
LAST_RESULTS = None  # BassKernelResults of the most recent run (for test harness)


def _install_ntff_hook():
    """Register the ctypes NTFF profiling hook under antenv.axon_hooks so
    run_bass_kernel_spmd(trace=True) can capture device profiles under axon.
    No-op if already present or the .so lacks the symbols."""
    import contextlib
    import ctypes
    import sys
    import types

    try:
        from antenv.axon_hooks import get_axon_ntff_profile_hook  # noqa: F401

        return True
    except ImportError:
        pass

    so_path = "/opt/axon/libaxon_pjrt.so"
    if not os.path.exists(so_path):
        return False
    lib = ctypes.CDLL(so_path)
    if not hasattr(lib, "axon_start_nrt_profile"):
        return False
    lib.axon_start_nrt_profile.argtypes = [
        ctypes.POINTER(ctypes.c_int64),
        ctypes.c_size_t,
    ]
    lib.axon_start_nrt_profile.restype = ctypes.c_int64
    lib.axon_stop_nrt_profile.argtypes = [ctypes.c_char_p]
    lib.axon_stop_nrt_profile.restype = ctypes.c_int64

    @contextlib.contextmanager
    def _hook(output_dir, device_ids):
        import jax

        jax.devices()
        if device_ids:
            ids = (ctypes.c_int64 * len(device_ids))(*device_ids)
            rc = lib.axon_start_nrt_profile(ids, len(device_ids))
        else:
            rc = lib.axon_start_nrt_profile(None, 0)
        if rc != 0:
            raise RuntimeError(f"axon_start_nrt_profile rc={rc}")
        try:
            yield
        finally:
            n = lib.axon_stop_nrt_profile(str(output_dir).encode())
            print(f"ntff profile: {n} file(s) written to {output_dir}")

    mod = types.ModuleType("antenv.axon_hooks")
    _state = {"hook": _hook}
    mod.set_axon_ntff_profile_hook = lambda h: _state.__setitem__("hook", h)
    mod.get_axon_ntff_profile_hook = lambda: _state["hook"]
    sys.modules["antenv.axon_hooks"] = mod
    import antenv

    antenv.axon_hooks = mod

    # artifact upload reaches for a cloud bucket that isn't available here
    bass_utils.upload_artifacts = lambda tmpdir: f"local:{tmpdir}"
    return True


def _build_nc():
    nc = bacc.Bacc(
        "TRN2", target_bir_lowering=False, debug=False, num_devices=N_CORES,
        enable_partition_id=False,
    )
    # x pre-tiled on host: xTp[q, j, p, m] = x.T[j*128+p, q*1024+m], so each
    # (q, j) DMA is a fully contiguous 256KB read with 2KB-per-partition
    # packets (1KB packets run the DMA queue ~40% slower).
    xTp = nc.dram_tensor(
        "xTp", [M_PAIRS, K_TILES, P, 2 * M_BLK], mybir.dt.bfloat16,
        kind="ExternalInput",
    )
    wn = nc.dram_tensor("wn", [N_SHARD, D_IN], mybir.dt.bfloat16, kind="ExternalInput")
    # w.T pre-tiled in k-tile pairs: wTt[jj, p, h*512+n] = w.T[(2*jj+h)*128+p, n]
    # so each DMA has 2KB-per-partition packets.
    wTt = nc.dram_tensor(
        "wTt", [K_TILES // 2, P, 2 * N_SHARD], mybir.dt.bfloat16,
        kind="ExternalInput",
    )
    outT = nc.dram_tensor(
        "outT", [N_SHARD, M_TOT], mybir.dt.float32, kind="ExternalOutput"
    )

    with tile.TileContext(nc) as tc, ExitStack() as ctx:
        spool = ctx.enter_context(tc.tile_pool(name="scales", bufs=1))
        wpool = ctx.enter_context(tc.tile_pool(name="wnat", bufs=2))
        wtpool = ctx.enter_context(tc.tile_pool(name="wtrans", bufs=6))
        sgpool = ctx.enter_context(tc.tile_pool(name="sign", bufs=1))
        xpool = ctx.enter_context(tc.tile_pool(name="xpair", bufs=2))
        opool = ctx.enter_context(tc.tile_pool(name="oblk", bufs=6))
        ppool = ctx.enter_context(tc.tile_pool(name="psum", bufs=8, space="PSUM"))

        # Queue assignment: sync = x loads + scale-weight loads (chained in
        # emission order so the FIFO queue is deterministic); scalar =
        # sign-weight loads, then output stores (which must wait on evictions
        # and would stall x loads).
        PAIR_W = 2 * M_BLK
        prev_sync_dma = [None]

        def sync_load(dst, src):
            dma = nc.sync.dma_start(dst, src)
            if prev_sync_dma[0] is not None:
                # add_dep_helper(waiter, dependency): this load is ordered
                # after the previous one on the sync queue.
                tile.add_dep_helper(
                    dma.ins, prev_sync_dma[0].ins, sync=False,
                    reason="sync DMA queue emission order",
                )
            prev_sync_dma[0] = dma
            return dma

        def issue_x_pair(q):
            xt = xpool.tile([P, K_TILES * PAIR_W], mybir.dt.bfloat16, tag="xpair")
            for j in range(K_TILES):
                sync_load(xt[:, j * PAIR_W : (j + 1) * PAIR_W], xTp[q, j, :, :])
            return xt

        def mm_block(pss, xt, b, ni, j):
            nc.tensor.matmul(
                pss[ni][:],
                S_all[:, j * N_SHARD + ni * P : j * N_SHARD + (ni + 1) * P],
                xt[:, j * PAIR_W + b * M_BLK : j * PAIR_W + b * M_BLK + M_BLK],
                start=(j == 0),
                stop=(j == K_TILES - 1),
            )

        def evict_block(pss, mb):
            # Evictions alternate between the scalar and vector engines so
            # the per-block eviction chain (and the kernel tail) is half as
            # long. Stores ride the scalar queue; for the final block the
            # sync queue (drained of x loads by then) takes half the store
            # triggers so the tail isn't serialized on one engine.
            last = mb == M_BLKS - 1
            for ni in range(N_TILES):
                ot = opool.tile([P, M_BLK], mybir.dt.float32, tag="ot", name="ot")
                dst = outT[ni * P : (ni + 1) * P, mb * M_BLK : (mb + 1) * M_BLK]
                if ni % 2 == 0:
                    nc.scalar.mul(ot[:], pss[ni][:], s_all[:, ni : ni + 1])
                else:
                    nc.vector.tensor_scalar_mul(
                        ot[:], pss[ni][:], s_all[:, ni : ni + 1]
                    )
                if last and ni % 2 == 1:
                    nc.sync.dma_start(dst, ot[:])
                else:
                    nc.scalar.dma_start(dst, ot[:])

        # Prologue: interleave sign-weight loads with the first x pair's
        # loads on the chained sync queue so the earliest matmuls are fed in
        # lockstep with minimal latency.
        S_all = sgpool.tile([P, K_TILES * N_SHARD], mybir.dt.bfloat16)
        xt0 = xpool.tile([P, K_TILES * PAIR_W], mybir.dt.bfloat16, tag="xpair")
        # Zero bias for the Sign activations as a plain SBUF tile (a float
        # bias would pull in a const-AP DRAM load during the preamble), and a
        # dummy 1-column sign to hoist the ACT LUT table load off the
        # critical path of the first real sign.
        zbias = spool.tile([P, 1], mybir.dt.float32)
        nc.gpsimd.memset(zbias[:], 0.0)
        nc.scalar.activation(
            S_all[:, 0:1], zbias[:], mybir.ActivationFunctionType.Sign,
            bias=zbias[:],
        )
        for jj in range(K_TILES // 2):
            wt_t = wtpool.tile([P, 2 * N_SHARD], mybir.dt.bfloat16)
            if jj == 0:
                # Split the first load/sign so the very first matmul (k-tile
                # 0, n-tile 0) is unblocked by a 32KB load + 128-col sign
                # instead of the full 512KB/1024-col pair, and slot the
                # first moving tile's load between the two sign chunks so
                # matmul #0's operands land back to back.
                sync_load(wt_t[:, 0:P], wTt[0, :, 0:P])
                nc.scalar.activation(
                    S_all[:, 0:P], wt_t[:, 0:P],
                    mybir.ActivationFunctionType.Sign, bias=zbias[:],
                )
                sync_load(xt0[:, 0:M_BLK], xTp[0, 0, :, 0:M_BLK])
                sync_load(wt_t[:, P:], wTt[0, :, P:])
                nc.scalar.activation(
                    S_all[:, P : 2 * N_SHARD], wt_t[:, P:],
                    mybir.ActivationFunctionType.Sign, bias=zbias[:],
                )
                sync_load(xt0[:, M_BLK:PAIR_W], xTp[0, 0, :, M_BLK:])
            else:
                sync_load(wt_t[:], wTt[jj, :, :])
                nc.scalar.activation(
                    S_all[:, 2 * jj * N_SHARD : (2 * jj + 2) * N_SHARD],
                    wt_t[:],
                    mybir.ActivationFunctionType.Sign,
                    bias=zbias[:],
                )
            for j in ((1,) if jj == 0 else (2 * jj, 2 * jj + 1)):
                sync_load(xt0[:, j * PAIR_W : (j + 1) * PAIR_W], xTp[0, j, :, :])

        # Per-out-feature scales: s[n] = mean_k |w[n, k]|, kept per n-tile as
        # a [128, 1] per-partition column (column i = n-tile i). Only needed
        # by the PSUM evictions (first one ~2 blocks in).
        # These ride the sync queue BEHIND the first x pair's loads so they
        # don't steal HBM bandwidth during the HBM-paced startup; they still
        # land well before the first eviction needs them.
        s_all = spool.tile([P, N_TILES], mybir.dt.float32)
        for i in range(N_TILES):
            wtile = wpool.tile([P, D_IN], mybir.dt.bfloat16)
            sync_load(wtile[:], wn[i * P : (i + 1) * P, :])
            nc.vector.reduce_sum(
                s_all[:, i : i + 1],
                wtile[:],
                axis=mybir.AxisListType.X,
                apply_absolute_value=True,
            )
        nc.vector.tensor_scalar_mul(s_all[:], s_all[:], 1.0 / D_IN)

        # Main loop: out.T[n, m] = sum_k S[k, n] * xT[k, m], scaled by s[n].
        # Pair 0 is computed j-outer across BOTH blocks (8 PSUM banks) so the
        # PE keeps pace with the HBM-limited startup stream; later pairs run
        # block-at-a-time j-outer (4 banks ping-ponging with the previous
        # block's draining 4).
        for q in range(M_PAIRS):
            xt = xt0 if q == 0 else issue_x_pair(q)
            if q == 0:
                pss2 = [
                    [
                        ppool.tile(
                            [P, M_BLK], mybir.dt.float32, tag="ps",
                            name=f"ps_{b}_{ni}",
                        )
                        for ni in range(N_TILES)
                    ]
                    for b in range(2)
                ]
                for j in range(K_TILES):
                    for b in range(2):
                        for ni in range(N_TILES):
                            mm_block(pss2[b], xt, b, ni, j)
                for b in range(2):
                    evict_block(pss2[b], b)
            else:
                for b in range(2):
                    last_blk = q == M_PAIRS - 1 and b == 1
                    pss = [
                        ppool.tile(
                            [P, M_BLK], mybir.dt.float32, tag="ps", name=f"ps{ni}"
                        )
                        for ni in range(N_TILES)
                    ]
                    if last_blk:
                        # ni-outer for the final block: each n-tile's stop
                        # matmul lands early, so its eviction + store overlap
                        # the remaining matmuls instead of serializing after
                        # the last one.
                        for ni in range(N_TILES):
                            for j in range(K_TILES):
                                mm_block(pss, xt, b, ni, j)
                    else:
                        for j in range(K_TILES):
                            for ni in range(N_TILES):
                                mm_block(pss, xt, b, ni, j)
                    evict_block(pss, 2 * q + b)

    nc.compile()
    return nc


def kernel(x, weight):
    global LAST_RESULTS
    nc = _CACHE.get("nc")
    if nc is None:
        nc = _CACHE["nc"] = _build_nc()

    x = np.asarray(x)
    weight = np.asarray(weight)
    orig_shape = x.shape

    # Host-side sharding/layout: xT in bf16 (replicated, pre-tiled so each
    # (pair, k-tile) chunk is contiguous), weight shard in both layouts.
    xT = x.reshape(M_TOT, D_IN).T  # [D_IN, M_TOT] view
    xTp = np.ascontiguousarray(
        xT.reshape(K_TILES, P, M_PAIRS, 2 * M_BLK)
        .transpose(2, 0, 1, 3)
        .astype(ml_dtypes.bfloat16)
    )  # [M_PAIRS, K_TILES, P, 1024]
    wt_full = np.ascontiguousarray(weight.T)  # [D_IN, D_OUT] f32
    in_maps = []
    for c in range(N_CORES):
        in_maps.append(
            {
                "xTp": xTp,
                "wn": np.ascontiguousarray(
                    weight[c * N_SHARD : (c + 1) * N_SHARD, :].astype(
                        ml_dtypes.bfloat16
                    )
                ),
                "wTt": np.ascontiguousarray(
                    wt_full[:, c * N_SHARD : (c + 1) * N_SHARD]
                    .reshape(K_TILES // 2, 2, P, N_SHARD)
                    .transpose(0, 2, 1, 3)
                    .reshape(K_TILES // 2, P, 2 * N_SHARD)
                    .astype(ml_dtypes.bfloat16)
                ),
            }
        )

    trace = bool(int(os.environ.get("BITLIN_TRACE", "0")))
    if trace:
        trace = _install_ntff_hook()
        base = os.environ.get("BITLIN_TRACE_DIR") or None
        if base:
            import tempfile

            os.makedirs(base, exist_ok=True)
            tmpdir = tempfile.mkdtemp(dir=base)
        else:
            tmpdir = None
    else:
        tmpdir = None
    res = bass_utils.run_bass_kernel_spmd(
        nc, in_maps, core_ids=list(range(N_CORES)), trace=trace, tmpdir=tmpdir
    )
    LAST_RESULTS = res

    outT_full = np.concatenate(
        [np.asarray(res.results[c]["outT"]) for c in range(N_CORES)], axis=0
    )  # [D_OUT, M_TOT] f32
    out = np.ascontiguousarray(outT_full.T).reshape(orig_shape).astype(np.float32)
    return out



# revision 2
# speedup vs baseline: 1.4445x; 1.4445x over previous
"""BitLinear kernel for Trainium2 (8 NeuronCores, tensor-parallel).

Computes: out = x @ (sign(w) * mean(|w|, axis=1, keepdims=True)).T
  x      : [4, 2048, 4096] f32
  weight : [4096, 4096] f32
  out    : [4, 2048, 4096] f32

Strategy (per sharding hint): shard weight rows (out features) 8-way.
Hybrid-precision contraction: the first K16 k-tiles run in bf16
(128-deep per matmul), the remaining 2*F8P k-tiles run as e4m3 fp8
DoubleRow pairs (256-deep per matmul at ~2x the bf16 PE rate).  Signs
are exactly representable in both dtypes, so the only extra error is
the e4m3 quantization of the x slice routed through fp8:
rel_l2 ~= 0.0265 * sqrt(2*F8P/32), ~1.75e-2 at F8P=7 (gate: 2e-2).

Each core:
  - receives x.T pre-tiled on host: bf16 k-tiles as 256KB chunks
    (xTp16), fp8 k-tile pairs in DoubleRow slot layout (x8p, 2KB per
    partition per chunk); its weight shard in bf16 natural layout (wn,
    for the per-row abs-mean scales) and k-tile-paired transposed
    layout (wTt, the sign operand source).
  - binarizes on device: Sign activation -> bf16 S_all for the bf16
    k-tiles and fp8 S8_all for the DoubleRow pairs; s = mean|w| via
    DVE reduction in f32.
  - matmuls: per 512-token block and 128-feature n-tile, K16 bf16
    matmuls then F8P DoubleRow matmuls accumulate one PSUM bank; the
    f32 per-feature scale is applied while evicting PSUM -> SBUF;
    stores write the feature-major shard outT [512, 8192].
Host gathers the 8 outT shards -> [4096, 8192] -> transpose -> out.
"""

import os
from contextlib import ExitStack

import numpy as np
import ml_dtypes

import concourse.bass as bass
import concourse.mybir as mybir
import concourse.tile as tile
from concourse import bacc, bass_utils

P = 128                 # SBUF partitions / PE array dim
D_IN = 4096             # contraction dim (in features)
D_OUT = 4096            # out features
M_TOT = 8192            # tokens (4*2048)
N_CORES = 8
N_SHARD = D_OUT // N_CORES      # 512 out features per core
K_TILES = D_IN // P             # 32
M_BLK = 512                     # moving free dim per matmul
M_BLKS = M_TOT // M_BLK         # 16
M_PAIRS = M_BLKS // 2           # 8 (x is loaded in block pairs)
N_TILES = N_SHARD // P          # 4

# Hybrid precision split: k-tiles [0, K16) in bf16, [K16, 32) as fp8
# DoubleRow pairs.  K16 must be even (wTt pair layout).
K16 = int(os.environ.get("BITLIN_K16", "18"))
F8P = (K_TILES - K16) // 2
assert K16 % 2 == 0 and K16 + 2 * F8P == K_TILES

PAIR_W = 2 * M_BLK      # 1024 tokens per x block pair

_CACHE = {}
LAST_RESULTS = None  # BassKernelResults of the most recent run (for test harness)


def _install_ntff_hook():
    """Register the ctypes NTFF profiling hook under antenv.axon_hooks so
    run_bass_kernel_spmd(trace=True) can capture device profiles under axon.
    No-op if already present or the .so lacks the symbols."""
    import contextlib
    import ctypes
    import sys
    import types

    try:
        from antenv.axon_hooks import get_axon_ntff_profile_hook  # noqa: F401

        return True
    except ImportError:
        pass

    so_path = "/opt/axon/libaxon_pjrt.so"
    if not os.path.exists(so_path):
        return False
    lib = ctypes.CDLL(so_path)
    if not hasattr(lib, "axon_start_nrt_profile"):
        return False
    lib.axon_start_nrt_profile.argtypes = [
        ctypes.POINTER(ctypes.c_int64),
        ctypes.c_size_t,
    ]
    lib.axon_start_nrt_profile.restype = ctypes.c_int64
    lib.axon_stop_nrt_profile.argtypes = [ctypes.c_char_p]
    lib.axon_stop_nrt_profile.restype = ctypes.c_int64

    @contextlib.contextmanager
    def _hook(output_dir, device_ids):
        import jax

        jax.devices()
        if device_ids:
            ids = (ctypes.c_int64 * len(device_ids))(*device_ids)
            rc = lib.axon_start_nrt_profile(ids, len(device_ids))
        else:
            rc = lib.axon_start_nrt_profile(None, 0)
        if rc != 0:
            raise RuntimeError(f"axon_start_nrt_profile rc={rc}")
        try:
            yield
        finally:
            n = lib.axon_stop_nrt_profile(str(output_dir).encode())
            print(f"ntff profile: {n} file(s) written to {output_dir}")

    mod = types.ModuleType("antenv.axon_hooks")
    _state = {"hook": _hook}
    mod.set_axon_ntff_profile_hook = lambda h: _state.__setitem__("hook", h)
    mod.get_axon_ntff_profile_hook = lambda: _state["hook"]
    sys.modules["antenv.axon_hooks"] = mod
    import antenv

    antenv.axon_hooks = mod

    # artifact upload reaches for a cloud bucket that isn't available here
    bass_utils.upload_artifacts = lambda tmpdir: f"local:{tmpdir}"
    return True


def _build_nc():
    nc = bacc.Bacc(
        "TRN2", target_bir_lowering=False, debug=False, num_devices=N_CORES,
        enable_partition_id=False,
    )
    # bf16 x k-tiles pre-tiled on host: xTp16[q, j, p, m] = x.T[j*128+p,
    # q*1024+m], so each (q, j) DMA is a fully contiguous 256KB read with
    # 2KB-per-partition packets.
    xTp16 = nc.dram_tensor(
        "xTp16", [M_PAIRS, K16, P, PAIR_W], mybir.dt.bfloat16,
        kind="ExternalInput",
    )
    # fp8 x DoubleRow pairs: x8p[q, jf, p, h*1024 + m] = x.T[(K16+2*jf+h)
    # *128+p, q*1024+m] in e4m3; each (q, jf) DMA is a contiguous 256KB
    # read with 2KB-per-partition packets.
    x8p = nc.dram_tensor(
        "x8p", [M_PAIRS, max(F8P, 1), P, 2 * PAIR_W], mybir.dt.float8e4,
        kind="ExternalInput",
    )
    wn = nc.dram_tensor("wn", [N_SHARD, D_IN], mybir.dt.bfloat16, kind="ExternalInput")
    # w.T pre-tiled in k-tile pairs: wTt[jj, p, h*512+n] = w.T[(2*jj+h)*128+p, n]
    # so each DMA has 2KB-per-partition packets.
    wTt = nc.dram_tensor(
        "wTt", [K_TILES // 2, P, 2 * N_SHARD], mybir.dt.bfloat16,
        kind="ExternalInput",
    )
    outT = nc.dram_tensor(
        "outT", [N_SHARD, M_TOT], mybir.dt.float32, kind="ExternalOutput"
    )

    with tile.TileContext(nc) as tc, ExitStack() as ctx:
        spool = ctx.enter_context(tc.tile_pool(name="scales", bufs=1))
        wpool = ctx.enter_context(tc.tile_pool(name="wnat", bufs=2))
        wtpool = ctx.enter_context(tc.tile_pool(name="wtrans", bufs=6))
        sgpool = ctx.enter_context(tc.tile_pool(name="sign", bufs=1))
        sg8pool = ctx.enter_context(tc.tile_pool(name="sign8", bufs=1))
        xpool = ctx.enter_context(tc.tile_pool(name="xpair", bufs=2))
        x8pool = ctx.enter_context(tc.tile_pool(name="x8pair", bufs=2))
        opool = ctx.enter_context(tc.tile_pool(name="oblk", bufs=6))
        ppool = ctx.enter_context(tc.tile_pool(name="psum", bufs=8, space="PSUM"))

        # Queue assignment: sync = x loads + scale-weight loads (chained in
        # emission order so the FIFO queue is deterministic); scalar =
        # sign-weight loads, then output stores (which must wait on evictions
        # and would stall x loads).
        prev_sync_dma = [None]

        def sync_load(dst, src):
            dma = nc.sync.dma_start(dst, src)
            if prev_sync_dma[0] is not None:
                # add_dep_helper(waiter, dependency): this load is ordered
                # after the previous one on the sync queue.
                tile.add_dep_helper(
                    dma.ins, prev_sync_dma[0].ins, sync=False,
                    reason="sync DMA queue emission order",
                )
            prev_sync_dma[0] = dma
            return dma

        def issue_x_pair(q):
            # bf16 tile [P, K16, 1024]; fp8 tile [P, 2*F8P, 1024] where the
            # middle dim (2*jf + h) is the DoubleRow slot dim per pair.
            xt = xpool.tile([P, K16, PAIR_W], mybir.dt.bfloat16, tag="xpair")
            x8t = x8pool.tile(
                [P, 2 * F8P, PAIR_W], mybir.dt.float8e4, tag="x8pair"
            )
            for j in range(K16):
                sync_load(xt[:, j, :], xTp16[q, j, :, :])
            for jf in range(F8P):
                sync_load(x8t[:, 2 * jf : 2 * jf + 2, :], x8p[q, jf, :, :])
            return xt, x8t

        def mm16(pss, xt, b, ni, j):
            nc.tensor.matmul(
                pss[ni][:],
                S_all[:, j * N_SHARD + ni * P : j * N_SHARD + (ni + 1) * P],
                xt[:, j, b * M_BLK : (b + 1) * M_BLK],
                start=(j == 0),
                stop=False,
            )

        def mm8(pss, x8t, b, ni, jf):
            nc.tensor.matmul(
                pss[ni][:],
                S8_all[:, 2 * jf : 2 * jf + 2, ni * P : (ni + 1) * P],
                x8t[:, 2 * jf : 2 * jf + 2, b * M_BLK : (b + 1) * M_BLK],
                start=False,
                stop=(jf == F8P - 1),
                perf_mode=mybir.MatmulPerfMode.DoubleRow,
            )

        def evict_block(pss, mb):
            # Evictions alternate between the scalar and vector engines so
            # the per-block eviction chain (and the kernel tail) is half as
            # long. Stores ride the scalar queue; for the final block the
            # sync queue (drained of x loads by then) takes half the store
            # triggers so the tail isn't serialized on one engine.
            last = mb == M_BLKS - 1
            for ni in range(N_TILES):
                ot = opool.tile([P, M_BLK], mybir.dt.float32, tag="ot", name="ot")
                dst = outT[ni * P : (ni + 1) * P, mb * M_BLK : (mb + 1) * M_BLK]
                if ni % 2 == 0:
                    nc.scalar.mul(ot[:], pss[ni][:], s_all[:, ni : ni + 1])
                else:
                    nc.vector.tensor_scalar_mul(
                        ot[:], pss[ni][:], s_all[:, ni : ni + 1]
                    )
                if last and ni % 2 == 1:
                    nc.sync.dma_start(dst, ot[:])
                else:
                    nc.scalar.dma_start(dst, ot[:])

        # Prologue: interleave sign-weight loads with the first x pair's
        # loads on the chained sync queue so the earliest matmuls are fed in
        # lockstep with minimal latency.
        S_all = sgpool.tile([P, K16 * N_SHARD], mybir.dt.bfloat16)
        S8_all = sg8pool.tile([P, 2 * F8P, N_SHARD], mybir.dt.float8e4)
        xt0 = xpool.tile([P, K16, PAIR_W], mybir.dt.bfloat16, tag="xpair")
        x8t0 = x8pool.tile([P, 2 * F8P, PAIR_W], mybir.dt.float8e4, tag="x8pair")
        # Zero bias for the Sign activations as a plain SBUF tile (a float
        # bias would pull in a const-AP DRAM load during the preamble), and a
        # dummy 1-column sign to hoist the ACT LUT table load off the
        # critical path of the first real sign.
        zbias = spool.tile([P, 1], mybir.dt.float32)
        nc.gpsimd.memset(zbias[:], 0.0)
        nc.scalar.activation(
            S_all[:, 0:1], zbias[:], mybir.ActivationFunctionType.Sign,
            bias=zbias[:],
        )
        for jj in range(K_TILES // 2):
            wt_t = wtpool.tile([P, 2 * N_SHARD], mybir.dt.bfloat16)
            is16 = jj < K16 // 2
            if jj == 0:
                # Split the first load/sign so the very first matmul (k-tile
                # 0, n-tile 0) is unblocked by a 32KB load + 128-col sign
                # instead of the full 512KB/1024-col pair, and slot the
                # first moving tile's load between the two sign chunks so
                # matmul #0's operands land back to back.
                sync_load(wt_t[:, 0:P], wTt[0, :, 0:P])
                nc.scalar.activation(
                    S_all[:, 0:P], wt_t[:, 0:P],
                    mybir.ActivationFunctionType.Sign, bias=zbias[:],
                )
                sync_load(xt0[:, 0, 0:M_BLK], xTp16[0, 0, :, 0:M_BLK])
                sync_load(wt_t[:, P:], wTt[0, :, P:])
                nc.scalar.activation(
                    S_all[:, P : 2 * N_SHARD], wt_t[:, P:],
                    mybir.ActivationFunctionType.Sign, bias=zbias[:],
                )
                sync_load(xt0[:, 0, M_BLK:PAIR_W], xTp16[0, 0, :, M_BLK:])
                sync_load(xt0[:, 1, :], xTp16[0, 1, :, :])
            elif is16:
                sync_load(wt_t[:], wTt[jj, :, :])
                nc.scalar.activation(
                    S_all[:, 2 * jj * N_SHARD : (2 * jj + 2) * N_SHARD],
                    wt_t[:],
                    mybir.ActivationFunctionType.Sign,
                    bias=zbias[:],
                )
                for j in (2 * jj, 2 * jj + 1):
                    sync_load(xt0[:, j, :], xTp16[0, j, :, :])
            else:
                # fp8 DoubleRow pair: sign straight into the e4m3 tile (the
                # scalar engine converts on output; +-1 is exact in e4m3).
                jf = jj - K16 // 2
                sync_load(wt_t[:], wTt[jj, :, :])
                nc.scalar.activation(
                    S8_all[:, 2 * jf : 2 * jf + 2, :],
                    wt_t[:],
                    mybir.ActivationFunctionType.Sign,
                    bias=zbias[:],
                )
                sync_load(x8t0[:, 2 * jf : 2 * jf + 2, :], x8p[0, jf, :, :])

        # Per-out-feature scales: s[n] = mean_k |w[n, k]|, kept per n-tile as
        # a [128, 1] per-partition column (column i = n-tile i). Only needed
        # by the PSUM evictions (first one ~2 blocks in).
        # These ride the sync queue BEHIND the first x pair's loads so they
        # don't steal HBM bandwidth during the HBM-paced startup; they still
        # land well before the first eviction needs them.
        s_all = spool.tile([P, N_TILES], mybir.dt.float32)
        for i in range(N_TILES):
            wtile = wpool.tile([P, D_IN], mybir.dt.bfloat16)
            sync_load(wtile[:], wn[i * P : (i + 1) * P, :])
            nc.vector.reduce_sum(
                s_all[:, i : i + 1],
                wtile[:],
                axis=mybir.AxisListType.X,
                apply_absolute_value=True,
            )
        nc.vector.tensor_scalar_mul(s_all[:], s_all[:], 1.0 / D_IN)

        # Main loop: out.T[n, m] = sum_k S[k, n] * xT[k, m], scaled by s[n].
        # Pair 0 is computed j-outer across BOTH blocks (8 PSUM banks) so the
        # PE keeps pace with the HBM-limited startup stream; later pairs run
        # block-at-a-time j-outer (4 banks ping-ponging with the previous
        # block's draining 4).
        for q in range(M_PAIRS):
            xt, x8t = (xt0, x8t0) if q == 0 else issue_x_pair(q)
            if q == 0:
                pss2 = [
                    [
                        ppool.tile(
                            [P, M_BLK], mybir.dt.float32, tag="ps",
                            name=f"ps_{b}_{ni}",
                        )
                        for ni in range(N_TILES)
                    ]
                    for b in range(2)
                ]
                for j in range(K16):
                    for b in range(2):
                        for ni in range(N_TILES):
                            mm16(pss2[b], xt, b, ni, j)
                for jf in range(F8P):
                    for b in range(2):
                        for ni in range(N_TILES):
                            mm8(pss2[b], x8t, b, ni, jf)
                for b in range(2):
                    evict_block(pss2[b], b)
            else:
                for b in range(2):
                    last_blk = q == M_PAIRS - 1 and b == 1
                    pss = [
                        ppool.tile(
                            [P, M_BLK], mybir.dt.float32, tag="ps", name=f"ps{ni}"
                        )
                        for ni in range(N_TILES)
                    ]
                    if last_blk:
                        # ni-outer for the final block: each n-tile's stop
                        # matmul lands early, so its eviction + store overlap
                        # the remaining matmuls instead of serializing after
                        # the last one.
                        for ni in range(N_TILES):
                            for j in range(K16):
                                mm16(pss, xt, b, ni, j)
                            for jf in range(F8P):
                                mm8(pss, x8t, b, ni, jf)
                    else:
                        for j in range(K16):
                            for ni in range(N_TILES):
                                mm16(pss, xt, b, ni, j)
                        for jf in range(F8P):
                            for ni in range(N_TILES):
                                mm8(pss, x8t, b, ni, jf)
                    evict_block(pss, 2 * q + b)

    nc.compile()
    return nc


def kernel(x, weight):
    global LAST_RESULTS
    nc = _CACHE.get("nc")
    if nc is None:
        nc = _CACHE["nc"] = _build_nc()

    x = np.asarray(x)
    weight = np.asarray(weight)
    orig_shape = x.shape

    # Host-side sharding/layout: xT pre-tiled (replicated): bf16 k-tiles
    # [0, K16), e4m3 DoubleRow pairs [K16, 32); weight shard in both layouts.
    xT = x.reshape(M_TOT, D_IN).T  # [D_IN, M_TOT] view
    xTk = xT.reshape(K_TILES, P, M_PAIRS, PAIR_W)  # [kt, p, q, m]
    xTp16 = np.ascontiguousarray(
        xTk[:K16].transpose(2, 0, 1, 3).astype(ml_dtypes.bfloat16)
    )  # [q, j, p, 1024]
    # [kt, p, q, m] for fp8 tiles -> [F8P, 2(h), p, q, m] -> [q, jf, p, h, m]
    x8p = np.ascontiguousarray(
        xTk[K16:]
        .reshape(F8P, 2, P, M_PAIRS, PAIR_W)
        .transpose(3, 0, 2, 1, 4)
        .reshape(M_PAIRS, F8P, P, 2 * PAIR_W)
        .astype(ml_dtypes.float8_e4m3)
    )
    wt_full = np.ascontiguousarray(weight.T)  # [D_IN, D_OUT] f32
    in_maps = []
    for c in range(N_CORES):
        in_maps.append(
            {
                "xTp16": xTp16,
                "x8p": x8p,
                "wn": np.ascontiguousarray(
                    weight[c * N_SHARD : (c + 1) * N_SHARD, :].astype(
                        ml_dtypes.bfloat16
                    )
                ),
                "wTt": np.ascontiguousarray(
                    wt_full[:, c * N_SHARD : (c + 1) * N_SHARD]
                    .reshape(K_TILES // 2, 2, P, N_SHARD)
                    .transpose(0, 2, 1, 3)
                    .reshape(K_TILES // 2, P, 2 * N_SHARD)
                    .astype(ml_dtypes.bfloat16)
                ),
            }
        )

    trace = bool(int(os.environ.get("BITLIN_TRACE", "0")))
    if trace:
        trace = _install_ntff_hook()
        base = os.environ.get("BITLIN_TRACE_DIR") or None
        if base:
            import tempfile

            os.makedirs(base, exist_ok=True)
            tmpdir = tempfile.mkdtemp(dir=base)
        else:
            tmpdir = None
    else:
        tmpdir = None
    res = bass_utils.run_bass_kernel_spmd(
        nc, in_maps, core_ids=list(range(N_CORES)), trace=trace, tmpdir=tmpdir
    )
    LAST_RESULTS = res

    outT_full = np.concatenate(
        [np.asarray(res.results[c]["outT"]) for c in range(N_CORES)], axis=0
    )  # [D_OUT, M_TOT] f32
    out = np.ascontiguousarray(outT_full.T).reshape(orig_shape).astype(np.float32)
    return out


# revision 5
# speedup vs baseline: 1.4700x; 1.0176x over previous
"""BitLinear kernel for Trainium2 (8 NeuronCores, tensor-parallel).

Computes: out = x @ (sign(w) * mean(|w|, axis=1, keepdims=True)).T
  x      : [4, 2048, 4096] f32
  weight : [4096, 4096] f32
  out    : [4, 2048, 4096] f32

Strategy (per sharding hint): shard weight rows (out features) 8-way.
Hybrid-precision contraction: the first K16 k-tiles run in bf16
(128-deep per matmul), the remaining 2*F8P k-tiles run as e4m3 fp8
DoubleRow pairs (256-deep per matmul at ~2x the bf16 PE rate).  Signs
are exactly representable in both dtypes, so the only extra error is
the e4m3 quantization of the x slice routed through fp8:
rel_l2 ~= 0.0265 * sqrt(2*F8P/32), ~1.75e-2 at F8P=7 (gate: 2e-2).

Each core:
  - receives x.T pre-tiled on host: bf16 k-tiles as 256KB chunks
    (xTp16), fp8 k-tile pairs in DoubleRow slot layout (x8p, 2KB per
    partition per chunk); its weight shard in bf16 natural layout (wn,
    for the per-row abs-mean scales) and k-tile-paired transposed
    layout (wTt, the sign operand source).
  - binarizes on device: Sign activation -> bf16 S_all for the bf16
    k-tiles and fp8 S8_all for the DoubleRow pairs; s = mean|w| via
    DVE reduction in f32.
  - matmuls: per 512-token block and 128-feature n-tile, K16 bf16
    matmuls then F8P DoubleRow matmuls accumulate one PSUM bank; the
    f32 per-feature scale is applied while evicting PSUM -> SBUF;
    stores write the feature-major shard outT [512, 8192].
Host gathers the 8 outT shards -> [4096, 8192] -> transpose -> out.
"""

import os
from contextlib import ExitStack

import numpy as np
import ml_dtypes

import concourse.bass as bass
import concourse.mybir as mybir
import concourse.tile as tile
from concourse import bacc, bass_utils

P = 128                 # SBUF partitions / PE array dim
D_IN = 4096             # contraction dim (in features)
D_OUT = 4096            # out features
M_TOT = 8192            # tokens (4*2048)
N_CORES = 8
N_SHARD = D_OUT // N_CORES      # 512 out features per core
K_TILES = D_IN // P             # 32
M_BLK = 512                     # moving free dim per matmul
M_BLKS = M_TOT // M_BLK         # 16
M_PAIRS = M_BLKS // 2           # 8 (x is loaded in block pairs)
N_TILES = N_SHARD // P          # 4

# Hybrid precision split: k-tiles [0, K16) in bf16, [K16, 32) as fp8
# DoubleRow pairs.  K16 must be even (wTt pair layout).
K16 = int(os.environ.get("BITLIN_K16", "18"))
F8P = (K_TILES - K16) // 2
assert K16 % 2 == 0 and K16 + 2 * F8P == K_TILES

PAIR_W = 2 * M_BLK      # 1024 tokens per x block pair

_CACHE = {}
LAST_RESULTS = None  # BassKernelResults of the most recent run (for test harness)


def _install_ntff_hook():
    """Register the ctypes NTFF profiling hook under antenv.axon_hooks so
    run_bass_kernel_spmd(trace=True) can capture device profiles under axon.
    No-op if already present or the .so lacks the symbols."""
    import contextlib
    import ctypes
    import sys
    import types

    try:
        from antenv.axon_hooks import get_axon_ntff_profile_hook  # noqa: F401

        return True
    except ImportError:
        pass

    so_path = "/opt/axon/libaxon_pjrt.so"
    if not os.path.exists(so_path):
        return False
    lib = ctypes.CDLL(so_path)
    if not hasattr(lib, "axon_start_nrt_profile"):
        return False
    lib.axon_start_nrt_profile.argtypes = [
        ctypes.POINTER(ctypes.c_int64),
        ctypes.c_size_t,
    ]
    lib.axon_start_nrt_profile.restype = ctypes.c_int64
    lib.axon_stop_nrt_profile.argtypes = [ctypes.c_char_p]
    lib.axon_stop_nrt_profile.restype = ctypes.c_int64

    @contextlib.contextmanager
    def _hook(output_dir, device_ids):
        import jax

        jax.devices()
        if device_ids:
            ids = (ctypes.c_int64 * len(device_ids))(*device_ids)
            rc = lib.axon_start_nrt_profile(ids, len(device_ids))
        else:
            rc = lib.axon_start_nrt_profile(None, 0)
        if rc != 0:
            raise RuntimeError(f"axon_start_nrt_profile rc={rc}")
        try:
            yield
        finally:
            n = lib.axon_stop_nrt_profile(str(output_dir).encode())
            print(f"ntff profile: {n} file(s) written to {output_dir}")

    mod = types.ModuleType("antenv.axon_hooks")
    _state = {"hook": _hook}
    mod.set_axon_ntff_profile_hook = lambda h: _state.__setitem__("hook", h)
    mod.get_axon_ntff_profile_hook = lambda: _state["hook"]
    sys.modules["antenv.axon_hooks"] = mod
    import antenv

    antenv.axon_hooks = mod

    # artifact upload reaches for a cloud bucket that isn't available here
    bass_utils.upload_artifacts = lambda tmpdir: f"local:{tmpdir}"
    return True


def _build_nc():
    nc = bacc.Bacc(
        "TRN2", target_bir_lowering=False, debug=False, num_devices=N_CORES,
        enable_partition_id=False,
    )
    # bf16 x k-tiles pre-tiled on host: xTp16[q, j, p, m] = x.T[j*128+p,
    # q*1024+m], so each (q, j) DMA is a fully contiguous 256KB read with
    # 2KB-per-partition packets.
    xTp16 = nc.dram_tensor(
        "xTp16", [M_PAIRS, K16, P, PAIR_W], mybir.dt.bfloat16,
        kind="ExternalInput",
    )
    # fp8 x DoubleRow pairs: x8p[q, jf, p, h*1024 + m] = x.T[(K16+2*jf+h)
    # *128+p, q*1024+m] in e4m3; each (q, jf) DMA is a contiguous 256KB
    # read with 2KB-per-partition packets.
    x8p = nc.dram_tensor(
        "x8p", [M_PAIRS, max(F8P, 1), P, 2 * PAIR_W], mybir.dt.float8e4,
        kind="ExternalInput",
    )
    wn = nc.dram_tensor("wn", [N_SHARD, D_IN], mybir.dt.bfloat16, kind="ExternalInput")
    # w.T pre-tiled in k-tile pairs: wTt[jj, p, h*512+n] = w.T[(2*jj+h)*128+p, n]
    # so each DMA has 2KB-per-partition packets.
    wTt = nc.dram_tensor(
        "wTt", [K_TILES // 2, P, 2 * N_SHARD], mybir.dt.bfloat16,
        kind="ExternalInput",
    )
    outT = nc.dram_tensor(
        "outT", [N_SHARD, M_TOT], mybir.dt.float32, kind="ExternalOutput"
    )

    with tile.TileContext(nc) as tc, ExitStack() as ctx:
        spool = ctx.enter_context(tc.tile_pool(name="scales", bufs=1))
        wpool = ctx.enter_context(tc.tile_pool(name="wnat", bufs=4))
        wtpool = ctx.enter_context(tc.tile_pool(name="wtrans", bufs=6))
        sgpool = ctx.enter_context(tc.tile_pool(name="sign", bufs=1))
        sg8pool = ctx.enter_context(tc.tile_pool(name="sign8", bufs=1))
        xpool = ctx.enter_context(tc.tile_pool(name="xpair", bufs=2))
        x8pool = ctx.enter_context(tc.tile_pool(name="x8pair", bufs=2))
        opool = ctx.enter_context(tc.tile_pool(name="oblk", bufs=6))
        ppool = ctx.enter_context(tc.tile_pool(name="psum", bufs=8, space="PSUM"))

        # Queue assignment: sync = x loads + scale-weight loads (chained in
        # emission order so the FIFO queue is deterministic); scalar =
        # sign-weight loads, then output stores (which must wait on evictions
        # and would stall x loads).
        prev_sync_dma = [None]

        def sync_load(dst, src):
            dma = nc.sync.dma_start(dst, src)
            if prev_sync_dma[0] is not None:
                # add_dep_helper(waiter, dependency): this load is ordered
                # after the previous one on the sync queue.
                tile.add_dep_helper(
                    dma.ins, prev_sync_dma[0].ins, sync=False,
                    reason="sync DMA queue emission order",
                )
            prev_sync_dma[0] = dma
            return dma

        def issue_x_pair(q):
            # bf16 tile [P, K16, 1024]; fp8 tile [P, 2*F8P, 1024] where the
            # middle dim (2*jf + h) is the DoubleRow slot dim per pair.
            xt = xpool.tile([P, K16, PAIR_W], mybir.dt.bfloat16, tag="xpair")
            x8t = x8pool.tile(
                [P, 2 * F8P, PAIR_W], mybir.dt.float8e4, tag="x8pair"
            )
            for j in range(K16):
                sync_load(xt[:, j, :], xTp16[q, j, :, :])
            for jf in range(F8P):
                sync_load(x8t[:, 2 * jf : 2 * jf + 2, :], x8p[q, jf, :, :])
            return xt, x8t

        def mm16(pss, xt, b, ni, j):
            nc.tensor.matmul(
                pss[ni][:],
                S_all[:, j * N_SHARD + ni * P : j * N_SHARD + (ni + 1) * P],
                xt[:, j, b * M_BLK : (b + 1) * M_BLK],
                start=(j == 0),
                stop=False,
            )

        def mm8(pss, x8t, b, ni, jf):
            nc.tensor.matmul(
                pss[ni][:],
                S8_all[:, 2 * jf : 2 * jf + 2, ni * P : (ni + 1) * P],
                x8t[:, 2 * jf : 2 * jf + 2, b * M_BLK : (b + 1) * M_BLK],
                start=False,
                stop=(jf == F8P - 1),
                perf_mode=mybir.MatmulPerfMode.DoubleRow,
            )

        def evict_block(pss, mb):
            # Evictions alternate between the scalar and vector engines so
            # the per-block eviction chain (and the kernel tail) is half as
            # long. Stores ride the scalar queue; for the final block the
            # sync queue (drained of x loads by then) takes half the store
            # triggers so the tail isn't serialized on one engine.
            last = mb == M_BLKS - 1
            for ni in range(N_TILES):
                ot = opool.tile([P, M_BLK], mybir.dt.float32, tag="ot", name="ot")
                dst = outT[ni * P : (ni + 1) * P, mb * M_BLK : (mb + 1) * M_BLK]
                if ni % 2 == 0:
                    nc.scalar.mul(ot[:], pss[ni][:], s_all[:, ni : ni + 1])
                else:
                    nc.vector.tensor_scalar_mul(
                        ot[:], pss[ni][:], s_all[:, ni : ni + 1]
                    )
                if last and ni % 2 == 1:
                    nc.sync.dma_start(dst, ot[:])
                else:
                    nc.scalar.dma_start(dst, ot[:])

        # Prologue: interleave sign-weight loads with the first x pair's
        # loads on the chained sync queue so the earliest matmuls are fed in
        # lockstep with minimal latency.
        S_all = sgpool.tile([P, K16 * N_SHARD], mybir.dt.bfloat16)
        S8_all = sg8pool.tile([P, 2 * F8P, N_SHARD], mybir.dt.float8e4)
        xt0 = xpool.tile([P, K16, PAIR_W], mybir.dt.bfloat16, tag="xpair")
        x8t0 = x8pool.tile([P, 2 * F8P, PAIR_W], mybir.dt.float8e4, tag="x8pair")
        # Zero bias for the Sign activations as a plain SBUF tile (a float
        # bias would pull in a const-AP DRAM load during the preamble), and a
        # dummy 1-column sign to hoist the ACT LUT table load off the
        # critical path of the first real sign.
        zbias = spool.tile([P, 1], mybir.dt.float32)
        nc.gpsimd.memset(zbias[:], 0.0)
        nc.scalar.activation(
            S_all[:, 0:1], zbias[:], mybir.ActivationFunctionType.Sign,
            bias=zbias[:],
        )
        for jj in range(K_TILES // 2):
            wt_t = wtpool.tile([P, 2 * N_SHARD], mybir.dt.bfloat16)
            is16 = jj < K16 // 2
            if jj == 0:
                # Split the first load/sign so the very first matmul (k-tile
                # 0, n-tile 0) is unblocked by a 32KB load + 128-col sign
                # instead of the full 512KB/1024-col pair, and slot the
                # first moving tile's load between the two sign chunks so
                # matmul #0's operands land back to back.
                sync_load(wt_t[:, 0:P], wTt[0, :, 0:P])
                nc.scalar.activation(
                    S_all[:, 0:P], wt_t[:, 0:P],
                    mybir.ActivationFunctionType.Sign, bias=zbias[:],
                )
                sync_load(xt0[:, 0, 0:M_BLK], xTp16[0, 0, :, 0:M_BLK])
                sync_load(wt_t[:, P:], wTt[0, :, P:])
                nc.scalar.activation(
                    S_all[:, P : 2 * N_SHARD], wt_t[:, P:],
                    mybir.ActivationFunctionType.Sign, bias=zbias[:],
                )
                sync_load(xt0[:, 0, M_BLK:PAIR_W], xTp16[0, 0, :, M_BLK:])
                sync_load(xt0[:, 1, :], xTp16[0, 1, :, :])
            elif is16:
                sync_load(wt_t[:], wTt[jj, :, :])
                nc.scalar.activation(
                    S_all[:, 2 * jj * N_SHARD : (2 * jj + 2) * N_SHARD],
                    wt_t[:],
                    mybir.ActivationFunctionType.Sign,
                    bias=zbias[:],
                )
                for j in (2 * jj, 2 * jj + 1):
                    sync_load(xt0[:, j, :], xTp16[0, j, :, :])
            else:
                # fp8 DoubleRow pair: sign straight into the e4m3 tile (the
                # scalar engine converts on output; +-1 is exact in e4m3).
                jf = jj - K16 // 2
                sync_load(wt_t[:], wTt[jj, :, :])
                nc.scalar.activation(
                    S8_all[:, 2 * jf : 2 * jf + 2, :],
                    wt_t[:],
                    mybir.ActivationFunctionType.Sign,
                    bias=zbias[:],
                )
                sync_load(x8t0[:, 2 * jf : 2 * jf + 2, :], x8p[0, jf, :, :])

        # Per-out-feature scales: s[n] = mean_k |w[n, k]|, kept per n-tile as
        # a [128, 1] per-partition column (column i = n-tile i). Only needed
        # by the PSUM evictions (first one ~2 blocks in).
        # These ride the SCALAR dma queue (idle until the first output store)
        # so the sync queue chain runs pair-0 x -> pair-1 x back to back; if
        # the scale pipeline sat between them, pair 1 would arrive ~20us
        # late, the PE would idle >3.4us, and HAM would re-throttle the clock
        # to 1.2 GHz for the restart.
        s_all = spool.tile([P, N_TILES], mybir.dt.float32)
        for i in range(N_TILES):
            wtile = wpool.tile([P, D_IN], mybir.dt.bfloat16)
            nc.scalar.dma_start(wtile[:], wn[i * P : (i + 1) * P, :])
            nc.vector.reduce_sum(
                s_all[:, i : i + 1],
                wtile[:],
                axis=mybir.AxisListType.X,
                apply_absolute_value=True,
            )
        nc.vector.tensor_scalar_mul(s_all[:], s_all[:], 1.0 / D_IN)

        # Main loop: out.T[n, m] = sum_k S[k, n] * xT[k, m], scaled by s[n].
        # Pair 0 is computed j-outer across BOTH blocks (8 PSUM banks) so the
        # PE keeps pace with the HBM-limited startup stream; later pairs run
        # block-at-a-time j-outer (4 banks ping-ponging with the previous
        # block's draining 4).
        for q in range(M_PAIRS):
            xt, x8t = (xt0, x8t0) if q == 0 else issue_x_pair(q)
            if q == 0:
                pss2 = [
                    [
                        ppool.tile(
                            [P, M_BLK], mybir.dt.float32, tag="ps",
                            name=f"ps_{b}_{ni}",
                        )
                        for ni in range(N_TILES)
                    ]
                    for b in range(2)
                ]
                for j in range(K16):
                    for b in range(2):
                        for ni in range(N_TILES):
                            mm16(pss2[b], xt, b, ni, j)
                # fp8 tail of the warmup runs b-outer so block 0's stop
                # matmuls + eviction land ~7us early, freeing its PSUM banks
                # before pair 1's first accumulation needs them.
                for b in range(2):
                    for jf in range(F8P):
                        for ni in range(N_TILES):
                            mm8(pss2[b], x8t, b, ni, jf)
                    evict_block(pss2[b], b)
            else:
                for b in range(2):
                    last_blk = q == M_PAIRS - 1 and b == 1
                    pss = [
                        ppool.tile(
                            [P, M_BLK], mybir.dt.float32, tag="ps", name=f"ps{ni}"
                        )
                        for ni in range(N_TILES)
                    ]
                    if last_blk:
                        # ni-outer for the final block: each n-tile's stop
                        # matmul lands early, so its eviction + store overlap
                        # the remaining matmuls instead of serializing after
                        # the last one.
                        for ni in range(N_TILES):
                            for j in range(K16):
                                mm16(pss, xt, b, ni, j)
                            for jf in range(F8P):
                                mm8(pss, x8t, b, ni, jf)
                    else:
                        for j in range(K16):
                            for ni in range(N_TILES):
                                mm16(pss, xt, b, ni, j)
                        for jf in range(F8P):
                            for ni in range(N_TILES):
                                mm8(pss, x8t, b, ni, jf)
                    evict_block(pss, 2 * q + b)

    nc.compile()
    return nc


def kernel(x, weight):
    global LAST_RESULTS
    nc = _CACHE.get("nc")
    if nc is None:
        nc = _CACHE["nc"] = _build_nc()

    x = np.asarray(x)
    weight = np.asarray(weight)
    orig_shape = x.shape

    # Host-side sharding/layout: xT pre-tiled (replicated): bf16 k-tiles
    # [0, K16), e4m3 DoubleRow pairs [K16, 32); weight shard in both layouts.
    xT = x.reshape(M_TOT, D_IN).T  # [D_IN, M_TOT] view
    xTk = xT.reshape(K_TILES, P, M_PAIRS, PAIR_W)  # [kt, p, q, m]
    xTp16 = np.ascontiguousarray(
        xTk[:K16].transpose(2, 0, 1, 3).astype(ml_dtypes.bfloat16)
    )  # [q, j, p, 1024]
    # [kt, p, q, m] for fp8 tiles -> [F8P, 2(h), p, q, m] -> [q, jf, p, h, m]
    x8p = np.ascontiguousarray(
        xTk[K16:]
        .reshape(F8P, 2, P, M_PAIRS, PAIR_W)
        .transpose(3, 0, 2, 1, 4)
        .reshape(M_PAIRS, F8P, P, 2 * PAIR_W)
        .astype(ml_dtypes.float8_e4m3)
    )
    wt_full = np.ascontiguousarray(weight.T)  # [D_IN, D_OUT] f32
    in_maps = []
    for c in range(N_CORES):
        in_maps.append(
            {
                "xTp16": xTp16,
                "x8p": x8p,
                "wn": np.ascontiguousarray(
                    weight[c * N_SHARD : (c + 1) * N_SHARD, :].astype(
                        ml_dtypes.bfloat16
                    )
                ),
                "wTt": np.ascontiguousarray(
                    wt_full[:, c * N_SHARD : (c + 1) * N_SHARD]
                    .reshape(K_TILES // 2, 2, P, N_SHARD)
                    .transpose(0, 2, 1, 3)
                    .reshape(K_TILES // 2, P, 2 * N_SHARD)
                    .astype(ml_dtypes.bfloat16)
                ),
            }
        )

    trace = bool(int(os.environ.get("BITLIN_TRACE", "0")))
    if trace:
        trace = _install_ntff_hook()
        base = os.environ.get("BITLIN_TRACE_DIR") or None
        if base:
            import tempfile

            os.makedirs(base, exist_ok=True)
            tmpdir = tempfile.mkdtemp(dir=base)
        else:
            tmpdir = None
    else:
        tmpdir = None
    res = bass_utils.run_bass_kernel_spmd(
        nc, in_maps, core_ids=list(range(N_CORES)), trace=trace, tmpdir=tmpdir
    )
    LAST_RESULTS = res

    outT_full = np.concatenate(
        [np.asarray(res.results[c]["outT"]) for c in range(N_CORES)], axis=0
    )  # [D_OUT, M_TOT] f32
    out = np.ascontiguousarray(outT_full.T).reshape(orig_shape).astype(np.float32)
    return out
